# revision 18
# baseline (speedup 1.0000x reference)
"""Multi-head attention (B=4, S=2048, D=768, H=12) on 8 Trainium2 cores.

Sharding: the 48 (batch, head) pairs are data-parallel; each core gets 6.
Per head on one core (all matmuls bf16, fp32 PSUM accumulation):
  x_aug [65, S]   : x^T with a ones row appended; QKV biases fold into the
                    projection matmuls (K=65 contraction), so PSUM->SBUF
                    moves are pure copies.
  QT/KT [128, S]  : q/k duplicated into both partition halves via col-tiled
                    projection pairs; enables row-tiled score pairs.
  scoresT [k, q]  : per kc-pair, tA/tB [128, 1024] PSUM tiles produced by
                    interleaved A/B matmuls on disjoint PE row halves
                    (tile_position (0,0)/(64,0)) so the HW overlaps them.
  P = exp(s/8)    : split across TWO engines: tA -> ACT (true exp, bf16),
                    tB -> DVE bit-trick exp (one tensor_scalar computing
                    round(s*16*log2e + (16256-C)) into uint16 = the bf16
                    bit pattern of 2^(s/(8 ln2)); negative saturation gives
                    +0.0, i.e. clean underflow). Halves the exp wall.
  V     [S, 64+1] : ones column appended -> AV matmul also produces the
                    softmax denominator.
  AV out [65, 512]: V_aug^T P accumulated over 16 k-chunks per 512-q chunk.
  norm            : out^T chunks are PE-transposed ([65,128] -> [128,65]
                    PSUM) so the denominator lands as a per-partition
                    column; one DVE reciprocal + per-partition tensor_scalar
                    multiply normalizes. No DRAM bounce, no serialized tail.
  out [S, 64] fp32 per head, gathered host-side with no transpose.

Scheduling: score-pair emission is interleaved with filler PE work (AV of
the previous block, QKV of the next head) via a FIFO of generators, keeping
PE dense while ACT/DVE drain the exp stream.

Other notes: q/k projections are computed once (col-tiled q|k pair, M=64
each) and the duplicate partition half is made by an SBUF->SBUF DMA; the
V projection runs as one 16-matmul burst into a single PSUM tile; the
transpose/normalize path runs in bf16 (PSUM bf16 slices need 4B-aligned
offsets, hence the 66-element chunk stride). Measured ~247-250us on HW
(baseline 294.6us), rel err ~1.3e-2 vs the 2e-2 gate.
"""

import sys
from collections import deque

for _p in ("/opt/trn_rl_repo",):
    if _p not in sys.path:
        sys.path.insert(0, _p)

import numpy as np

B, S, D, H = 4, 2048, 768, 12
DH = 64
NCORES = 8
HPC = (B * H) // NCORES  # 6 heads per core
SCALE = 1.0 / 8.0
NKC = S // 128  # 16 k-chunks
NQB = 2  # q blocks of 1024
QB = S // NQB
PUMPS_PER_PAIR = 4
# DVE bit-trick exp constants: bits = round(s * 16*log2e + (16256 - C))
EXP_C = 5.8
DVE_A = float(16.0 / np.log(2.0))
DVE_B = float(127.0 * 128.0 - EXP_C)


def _split_multi_waits(nc):
    """This walrus build rejects >1 sync wait per instruction. Insert
    single-wait NoOps (same engine, so same instruction stream) ahead of
    any instruction carrying several waits."""
    import bass_rust
    import concourse.mybir as mybir

    n_split = 0
    for f in nc.m.functions:
        for bb in f.blocks:
            out = []
            dirty = False
            for inst in bb.instructions:
                si = inst.sync_info
                if si is not None and len(si.on_wait) > 1:
                    waits = list(si.on_wait)
                    for j, w in enumerate(waits[:-1]):
                        nop = mybir.InstNoOp(name=f"{inst.name}-w{j}", ins=[], outs=[])
                        nop.engine = inst.engine
                        nop.sync_info = bass_rust.SyncInfo(on_wait=[w], on_update=[])
                        out.append(nop)
                    si.on_wait = waits[-1:]
                    dirty = True
                    n_split += 1
                out.append(inst)
            if dirty:
                bb.instructions = out
    return n_split


_BUILT = None


def build():
    global _BUILT
    if _BUILT is not None:
        return _BUILT
    import concourse.bass as bass
    import concourse.mybir as mybir
    import concourse.tile as tile

    F32 = mybir.dt.float32
    BF = mybir.dt.bfloat16
    U16 = mybir.dt.uint16
    AF = mybir.ActivationFunctionType
    ALU = mybir.AluOpType

    nc = bass.Bass()
    xTd = nc.dram_tensor("xT", [HPC, 65, S], BF, kind="ExternalInput")
    wqkd = nc.dram_tensor("wqk", [HPC, 65, 2, 64], BF, kind="ExternalInput")
    wvTd = nc.dram_tensor("wvT", [HPC, 65, 64], BF, kind="ExternalInput")
    identd = nc.dram_tensor("ident", [65, 65], BF, kind="ExternalInput")
    outd = nc.dram_tensor("out", [HPC, S, 64], F32, kind="ExternalOutput")

    with tile.TileContext(nc) as tc:
        with (
            tc.tile_pool(name="const", bufs=1) as cpool,
            tc.tile_pool(name="x", bufs=2) as xpool,
            tc.tile_pool(name="w", bufs=2) as wpool,
            tc.tile_pool(name="qk", bufs=2) as qkpool,
            tc.tile_pool(name="v", bufs=2) as vpool,
            tc.tile_pool(name="pt", bufs=2 * NKC * NQB) as ptpool,
            tc.tile_pool(name="ot", bufs=6) as otpool,
            tc.tile_pool(name="r", bufs=4) as rpool,
            tc.tile_pool(name="ob", bufs=6) as opool,
            tc.tile_pool(name="sp", bufs=3, space="PSUM") as sppool,
            tc.tile_pool(name="avp", bufs=2, space="PSUM") as avpool,
        ):
            ident = cpool.tile([65, 65], BF, tag="id")
            nc.sync.dma_start(ident[:], identd[:])

            state = {}

            def qkv_steps(i):
                x_t = xpool.tile([65, S], BF, tag="x", name=f"x{i}")
                for xc in range(4):
                    nc.gpsimd.dma_start(
                        x_t[:, xc * 512 : (xc + 1) * 512],
                        xTd[i, :, xc * 512 : (xc + 1) * 512],
                    )
                w_t = wpool.tile([65, 2, 64], BF, tag="wqk", name=f"wqk{i}")
                nc.gpsimd.dma_start(w_t[:], wqkd[i])
                wv_t = wpool.tile([65, 64], BF, tag="wv", name=f"wv{i}")
                nc.gpsimd.dma_start(wv_t[:], wvTd[i])
                yield

                # V with ones column (denominator trick); bias folded.
                # Runs first: it has no qt/kt pool deps, so it provides
                # early PE filler (esp. for the head-0 ramp).
                v_sb = vpool.tile([128, NKC, 65], BF, tag="v", name=f"v{i}")
                nc.vector.memset(v_sb[:, :, 64:65], 1.0)
                ps_v = sppool.tile([128, NKC, 64], F32, tag="sp", name=f"vp{i}")
                for c in range(NKC):
                    nc.tensor.matmul(
                        ps_v[:, c, :],
                        x_t[0:65, c * 128 : (c + 1) * 128],
                        wv_t[:],
                    )
                    if c % 8 == 7:
                        yield
                nc.vector.tensor_copy(v_sb[:, :, 0:64], ps_v[:])
                yield

                # QT/KT duplicated into both partition halves (col-tiled
                # concurrent pair); bias folded via the ones row (K=65).
                qt = qkpool.tile([128, S], BF, tag="qt", name=f"qt{i}")
                kt = qkpool.tile([128, S], BF, tag="kt", name=f"kt{i}")
                for qm in range(4):
                    ps = sppool.tile([128, 512], F32, tag="sp", name=f"qk{i}_{qm}")
                    rhs = x_t[0:65, qm * 512 : (qm + 1) * 512]
                    nc.tensor.matmul(
                        ps[0:64, :], w_t[:, 0, :], rhs, tile_position=(0, 0)
                    )
                    nc.tensor.matmul(
                        ps[64:128, :], w_t[:, 1, :], rhs, tile_position=(0, 64)
                    )
                    nc.scalar.copy(qt[0:64, qm * 512 : (qm + 1) * 512], ps[0:64, :])
                    nc.scalar.copy(kt[0:64, qm * 512 : (qm + 1) * 512], ps[64:128, :])
                    yield
                nc.sync.dma_start(qt[64:128, :], qt[0:64, :])
                nc.sync.dma_start(kt[64:128, :], kt[0:64, :])
                yield
                state[i] = {"qt": qt, "kt": kt, "v": v_sb, "pt": {}}

            def sc_pair(i, jb, kc):
                """One kc-pair of row-tiled score matmuls, A/B interleaved so
                the PE overlaps them; exp split across ACT (tA) / DVE (tB)."""
                qt, kt = state[i]["qt"], state[i]["kt"]
                pt = state[i]["pt"].setdefault(jb, [None] * NKC)
                tA = sppool.tile([128, QB], F32, tag="sp", name=f"sA{i}_{jb}_{kc}")
                tB = sppool.tile([128, QB], F32, tag="sp", name=f"sB{i}_{jb}_{kc}")
                lA = kt[0:64, kc * 128 : (kc + 1) * 128]
                lB = kt[64:128, (kc + 8) * 128 : (kc + 9) * 128]
                for qm in range(QB // 512):
                    q0 = jb * QB + qm * 512
                    sl = slice(qm * 512, (qm + 1) * 512)
                    nc.tensor.matmul(
                        tA[:, sl], lA, qt[0:64, q0 : q0 + 512], tile_position=(0, 0)
                    )
                    nc.tensor.matmul(
                        tB[:, sl], lB, qt[64:128, q0 : q0 + 512], tile_position=(64, 0)
                    )
                pA = ptpool.tile([128, QB], BF, tag="pt", name=f"pA{i}_{jb}_{kc}")
                nc.scalar.activation(pA[:], tA[:], AF.Exp, scale=SCALE)
                pB = ptpool.tile([128, QB], U16, tag="pt", name=f"pB{i}_{jb}_{kc}")
                nc.vector.tensor_scalar(pB[:], tB[:], DVE_A, DVE_B, ALU.mult, ALU.add)
                pt[kc] = (pA, False)
                pt[kc + 8] = (pB, True)

            def av_steps(i, jb):
                """Generator: AV matmuls per 512-q chunk; then PE-transpose
                the [65, 512] result so the denominator becomes a
                per-partition column; reciprocal + per-partition multiply
                normalizes; store [128, 64] fp32 tiles."""
                v_sb = state[i]["v"]
                pt = state[i]["pt"].pop(jb)
                ots_list = []
                for g in range(QB // 512):
                    av = avpool.tile([65, 512], F32, tag="av", name=f"av{i}_{jb}_{g}")
                    for kc in range(NKC):
                        t, is_u16 = pt[kc]
                        rhs = t[:, g * 512 : (g + 1) * 512]
                        if is_u16:
                            rhs = rhs.bitcast(mybir.dt.bfloat16)
                        nc.tensor.matmul(
                            av[:],
                            v_sb[:, kc, :],
                            rhs,
                            start=(kc == 0),
                            stop=(kc == NKC - 1),
                        )
                        if kc % 2 == 1:
                            yield
                    ots = otpool.tile([65, 512], BF, tag="ot", name=f"ot{i}_{jb}_{g}")
                    nc.scalar.copy(ots[:], av[:])
                    ots_list.append(ots)
                    yield
                for g, ots in enumerate(ots_list):
                    tp = avpool.tile([128, 4, 66], BF, tag="av", name=f"tp{i}_{jb}_{g}")
                    for c in range(4):
                        nc.tensor.transpose(
                            tp[:, c, 0:65], ots[0:65, c * 128 : (c + 1) * 128], ident[:]
                        )
                    r = rpool.tile([128, 4, 1], F32, tag="r", name=f"r{i}_{jb}_{g}")
                    nc.vector.reciprocal(r[:], tp[:, :, 64:65])
                    yield
                    q0 = jb * QB + g * 512
                    osb = opool.tile([128, 4, 64], F32, tag="ob", name=f"o{i}_{jb}_{g}")
                    for c in range(4):
                        nc.vector.tensor_scalar_mul(
                            osb[:, c, :], tp[:, c, 0:64], r[:, c, :]
                        )
                    nc.sync.dma_start(
                        outd[i, q0 : q0 + 512, :].rearrange(
                            "(c p) e -> p c e", c=4
                        ),
                        osb[:],
                    )
                    yield

            fillers = deque()

            def pump(n):
                while n > 0 and fillers:
                    try:
                        next(fillers[0])
                        n -= 1
                    except StopIteration:
                        fillers.popleft()

            def drain(gen=None):
                while fillers and (gen is None or gen in fillers):
                    pump(1)

            def unit(i, jb):
                for kc in range(NKC // 2):
                    sc_pair(i, jb, kc)
                    pump(PUMPS_PER_PAIR)

            # head 0 QKV runs eagerly; afterwards QKV(i+1) + AV trail the
            # score stream as interleaved filler, lagging by one q-block
            g0 = qkv_steps(0)
            fillers.append(g0)
            drain(g0)
            qg = qkv_steps(1)
            fillers.append(qg)
            unit(0, 0)
            for i in range(HPC):
                if i > 0:
                    fillers.append(av_steps(i - 1, 1))
                    unit(i, 0)
                fillers.append(av_steps(i, 0))
                unit(i, 1)
                if qg is not None:
                    drain(qg)
                qg = qkv_steps(i + 2) if i + 2 < HPC else None
                if qg is not None:
                    fillers.append(qg)
            fillers.append(av_steps(HPC - 1, 1))
            drain()

    _split_multi_waits(nc)
    _BUILT = nc
    return nc


def _core_inputs(sequences, wq, bq, wk, bk, wv, bv):
    import ml_dtypes

    bf16 = ml_dtypes.bfloat16
    xh = np.asarray(sequences, dtype=np.float32).reshape(B, S, H, DH)
    wq, bq = np.asarray(wq, np.float32), np.asarray(bq, np.float32)
    wk, bk = np.asarray(wk, np.float32), np.asarray(bk, np.float32)
    wv, bv = np.asarray(wv, np.float32), np.asarray(bv, np.float32)
    ident = np.eye(65).astype(ml_dtypes.bfloat16)
    in_maps = []
    for c in range(NCORES):
        xT = np.empty((HPC, 65, S), dtype=bf16)
        wqk = np.empty((HPC, 65, 2, 64), dtype=bf16)
        wvT = np.empty((HPC, 65, 64), dtype=bf16)
        for i in range(HPC):
            f = c * HPC + i
            b, h = f // H, f % H
            xT[i, 0:64] = np.ascontiguousarray(xh[b, :, h, :].T).astype(bf16)
            xT[i, 64] = np.ones(S, dtype=bf16)
            wqk[i, 0:64, 0, :] = wq[h].T.astype(bf16)
            wqk[i, 0:64, 1, :] = wk[h].T.astype(bf16)
            wqk[i, 64, 0, :] = bq[h].astype(bf16)
            wqk[i, 64, 1, :] = bk[h].astype(bf16)
            wvT[i, 0:64] = wv[h].T.astype(bf16)
            wvT[i, 64] = bv[h].astype(bf16)
        in_maps.append({"xT": xT, "wqk": wqk, "wvT": wvT, "ident": ident})
    return in_maps


def _gather(results):
    out = np.empty((B, S, H, DH), np.float32)
    for c in range(NCORES):
        o = np.asarray(results[c]["out"])  # [HPC, S, 64]
        for i in range(HPC):
            f = c * HPC + i
            b, h = f // H, f % H
            out[b, :, h, :] = o[i]
    return out.reshape(B, S, D)


def kernel(sequences, wq, bq, wk, bk, wv, bv):
    from concourse.bass_utils import run_bass_kernel_spmd

    nc = build()
    in_maps = _core_inputs(sequences, wq, bq, wk, bk, wv, bv)
    res = run_bass_kernel_spmd(nc, in_maps, list(range(NCORES)))
    return _gather(res.results)
